# revision 11
# baseline (speedup 1.0000x reference)
"""CRF-RNN 3D dense-CRF mean-field kernel for Trainium2, sharded over 8 NeuronCores.

Strategy (column-sharded kernel matrices, K-stationary narrow GEMMs):
- Each core owns 512 columns (voxels j) of the two 4096x4096 Gaussian kernel
  matrices, stored fp16 in SBUF (8.4 MB total).
- Filtering GEMM is re-associated so the K chunk is the PE-stationary side and
  the narrow (21-wide) q tile is the moving side: out[j,l] += K[i,j]^T q[i,l]
  accumulated over 32 i-chunks into PSUM, directly in (voxel x label) layout.
  8 sequential accumulation chains (2 kernels x 4 j-blocks) per iteration.
- The bilateral slice-normalization 1/sum_i K[i,j] is computed by feeding a
  ones vector through the same K-stationary path (1-wide outputs), landing
  per-partition with no DRAM bounce; the spatial normalizer is folded into
  the separable build tables on the host (1/Sz into gzt, 1/(Sy*Sx) scaling
  tyx after the exp), so K_sp comes out pre-normalized.
- Label mixing (A = C@W per kernel) happens post-GEMM on own columns only:
  per-chunk PE transposes stack both kernels' (j x l) outputs as (l x j) rows
  0:21 / 32:53 of a zero-padded [64, 512] tile, one stacked matmul applies
  [A_sp^T; A_bi^T] and sums the two kernels, and 4 small transposes return
  msg to (j x l); the bilateral norm rides the psum->sbuf copy as a
  broadcast multiply.
- Iteration 1's softmax runs on the full replicated fp16 unary locally, and
  its GEMM interleaves with the kernel build (per-chunk dependencies), so no
  all-gather is needed before iteration 2.
- Iterations 2-5: softmax in (j x l) layout (ACT exp + DVE reduce/reciprocal/
  broadcast-scale), AllGather of qT (4096x21 fp16) through HBM, gather DMAs
  spread across DGE queues.
"""

import os
from contextlib import ExitStack
import sys

sys.path.insert(0, "/opt/trn_rl_repo")

import numpy as np

import concourse.bass as bass
import concourse.tile as tile
from concourse import bacc, mybir
from concourse.bass_utils import run_bass_kernel_spmd

ALPHA, BETA, GAMMA = 67.0, 3.0, 1.0
NUM_ITERATIONS = 5
L = 21
C_IMG = 3
D = W = H = 16
N = D * W * H           # 4096
NCORES = 8
SH = N // NCORES        # 512 columns per core
NCH = SH // 128         # 4 local chunks (j-blocks)
GCH = N // 128          # 32 global chunks
FBI = 6 + 2             # bilateral features + [ones, -0.5|f|^2] augmentation

f32 = mybir.dt.float32
f16 = mybir.dt.float16
AF = mybir.ActivationFunctionType
ALU = mybir.AluOpType
X_AXIS = mybir.AxisListType.X

_CACHE = {}


def _build_program():
    """Emit the SPMD Bass program (identical for all 8 cores)."""
    nc = bacc.Bacc("TRN2", target_bir_lowering=False, debug=False,
                   num_devices=NCORES)

    ayx_d = nc.dram_tensor("ayx", [4, 256], f16, kind="ExternalInput").ap()
    byx_d = nc.dram_tensor("byx", [4, 256], f16, kind="ExternalInput").ap()
    gzt_d = nc.dram_tensor("gzt", [128, GCH, 2], f32, kind="ExternalInput").ap()
    rnyx_d = nc.dram_tensor("rnyx", [128, 256], f16, kind="ExternalInput").ap()
    a_bi_d = nc.dram_tensor("a_bi", [FBI, N], f16, kind="ExternalInput").ap()
    b_bi_d = nc.dram_tensor("b_bi", [FBI, SH], f16, kind="ExternalInput").ap()
    mixM_d = nc.dram_tensor("mixM", [64, L], f16, kind="ExternalInput").ap()
    ident_d = nc.dram_tensor("ident", [128, 128], f16,
                             kind="ExternalInput").ap()
    unT_d = nc.dram_tensor("unT", [SH, L], f32, kind="ExternalInput").ap()
    unTf_d = nc.dram_tensor("unTf", [N, L], f16, kind="ExternalInput").ap()
    outT_d = nc.dram_tensor("outT", [SH, L], f32, kind="ExternalOutput").ap()

    rg = [list(range(NCORES))]

    with tile.TileContext(nc) as tc:
        with (
            tc.tile_pool(name="const", bufs=1) as const,
            tc.tile_pool(name="kbig", bufs=1) as kbig,
            tc.tile_pool(name="work", bufs=3) as work,
            tc.tile_pool(name="qpool", bufs=2) as qpool,
            tc.tile_pool(name="dram", bufs=1, space="DRAM") as dram,
        ):
            # ---- load constants/input to SBUF ----
            # a_bi split so the first build matmuls start early
            a_bi_s = const.tile([FBI, N], f16)
            dmae = [nc.sync, nc.scalar, nc.gpsimd, nc.sync]
            for s in range(4):
                dmae[s].dma_start(out=a_bi_s[:, s * 1024:(s + 1) * 1024],
                                  in_=a_bi_d[:, s * 1024:(s + 1) * 1024])
            b_bi_s = const.tile([FBI, SH], f16)
            nc.scalar.dma_start(out=b_bi_s, in_=b_bi_d)
            ayx_s = const.tile([4, 256], f16)
            nc.sync.dma_start(out=ayx_s, in_=ayx_d)
            byx_s = const.tile([4, 256], f16)
            nc.sync.dma_start(out=byx_s, in_=byx_d)
            gzt_s = const.tile([128, GCH, 2], f32)
            nc.gpsimd.dma_start(out=gzt_s, in_=gzt_d)
            rnyx_s = const.tile([128, 256], f16)
            nc.sync.dma_start(out=rnyx_s, in_=rnyx_d)
            mixM_s = const.tile([64, L], f16)
            nc.scalar.dma_start(out=mixM_s, in_=mixM_d)
            ident_s = const.tile([128, 128], f16)
            nc.gpsimd.dma_start(out=ident_s, in_=ident_d)
            unT_s = const.tile([128, NCH, L], f32)
            nc.sync.dma_start(
                out=unT_s, in_=unT_d.rearrange("(c p) l -> p c l", p=128))
            # full transposed unary (all voxels, fp16): iteration 1's softmax
            # is computed locally on every core, skipping the first all-gather
            unTf_s = const.tile([128, GCH, L], f16)
            unTf_v = unTf_d.rearrange("(c p) l -> p c l", p=128)
            nc.sync.dma_start(out=unTf_s[:, 0:16, :], in_=unTf_v[:, 0:16, :])
            nc.scalar.dma_start(out=unTf_s[:, 16:32, :], in_=unTf_v[:, 16:32, :])
            ones_s = const.tile([128, 1], f16)
            nc.vector.memset(ones_s, 1.0)
            # persistent stacking tile for the mixing matmul; gap rows
            # (21:32, 53:64) are zeroed once and never written again
            sts = const.tile([64, SH], f16)
            nc.vector.memset(sts, 0.0)

            K_sp = kbig.tile([128, GCH, SH], f16)
            K_bi = kbig.tile([128, GCH, SH], f16)

            # ---- iteration 1 softmax on replicated unary (full voxel set) --
            ef = work.tile([128, GCH, L], f32, name="ef")
            nc.scalar.activation(ef, unTf_s, AF.Exp)
            sfull = work.tile([128, GCH], f32, name="sfull")
            nc.vector.reduce_sum(sfull, ef, axis=X_AXIS)
            rfull = work.tile([128, GCH], f32, name="rfull")
            nc.vector.reciprocal(rfull, sfull)
            qTf0 = qpool.tile([128, GCH, L], f16, name="qTf0", tag="qTf")
            nc.vector.tensor_tensor(
                qTf0, ef, rfull.unsqueeze(-1).broadcast_to([128, GCH, L]),
                op=ALU.mult)

            # ---- build kernel matrices ----
            build_ps = ExitStack()
            psum_build = build_ps.enter_context(
                tc.tile_pool(name="psum_build", bufs=2, space="PSUM"))

            # spatial: separable.  Tyx base tables via one matmul+exp, scaled
            # by the host-folded yx-normalizer, then K_sp = tyx * gz with the
            # z-normalizer folded into gzt on the host.
            ptyx = psum_build.tile([128, 2, 256], f32, tag="psb")
            for v in range(2):
                nc.tensor.matmul(
                    ptyx[:, v, :],
                    lhsT=ayx_s[:, v * 128:(v + 1) * 128],
                    rhs=byx_s[:],
                    start=True, stop=True)
            tyx = const.tile([128, 2, 256], f16)
            nc.scalar.activation(tyx, ptyx[:], AF.Exp)
            nc.vector.tensor_tensor(
                tyx, tyx, rnyx_s.unsqueeze(1).broadcast_to([128, 2, 256]),
                op=ALU.mult)
            # K_sp = tyx (x,y base) * gz (z factor): one broadcast
            # tensor_tensor per chunk parity, split across DVE and Pool
            for v, eng in ((0, nc.vector), (1, nc.gpsimd)):
                eng.tensor_tensor(
                    K_sp[:, v::2, :].rearrange("p c (h f) -> p c h f", h=2),
                    tyx[:, v, :].unsqueeze(1).unsqueeze(1).broadcast_to(
                        [128, GCH // 2, 2, 256]),
                    gzt_s[:, v::2, :].unsqueeze(-1).broadcast_to(
                        [128, GCH // 2, 2, 256]),
                    op=ALU.mult)

            # bilateral: dense augmented matmul + exp per 3-chunk group
            GRP = 3
            ic = 0
            while ic < GCH:
                g = min(GRP, GCH - ic)
                ps = psum_build.tile([128, GRP, SH], f32, tag="psb")
                for u in range(g):
                    nc.tensor.matmul(
                        ps[:, u, :],
                        lhsT=a_bi_s[:, (ic + u) * 128:(ic + u + 1) * 128],
                        rhs=b_bi_s[:],
                        start=True, stop=True)
                nc.scalar.activation(
                    K_bi[:, ic:ic + g, :], ps[:, 0:g, :], AF.Exp)
                ic += g
            build_ps.close()

            # bilateral norm via ones through the K-stationary path:
            # pn[j, 0] += K_bi[i, j]^T ones -> per-partition, no bounce
            rn_bi = const.tile([128, NCH], f32)
            with tc.tile_pool(name="psum_pn", bufs=1, space="PSUM") as psum_pn:
                pn = psum_pn.tile([128, NCH], f32)
                for jb in range(NCH):
                    for ic in range(GCH):
                        nc.tensor.matmul(
                            pn[:, jb:jb + 1],
                            lhsT=K_bi[:, ic, jb * 128:(jb + 1) * 128],
                            rhs=ones_s[:],
                            start=(ic == 0), stop=(ic == GCH - 1))
                nc.vector.reciprocal(rn_bi, pn[:])

            # ---- mean-field iterations ----
            it_ps = ExitStack()
            psum_gemm = it_ps.enter_context(
                tc.tile_pool(name="psum_gemm", bufs=1, space="PSUM"))
            psum_mix = it_ps.enter_context(
                tc.tile_pool(name="psum_mix", bufs=1, space="PSUM"))

            cur_q = qTf0
            for it in range(NUM_ITERATIONS):
                # filtering GEMMs: 8 sequential accumulation chains of
                # 32 chunk matmuls each, 21-wide moving side.  The mixing
                # copies/transposes for chain k are interleaved between
                # later chains so only the last chain's tail is exposed.
                pm = {}
                for key, K_s in (("sp", K_sp), ("bi", K_bi)):
                    pm[key] = psum_gemm.tile([128, NCH, L], f32,
                                             name=f"pm_{key}_{it}", tag="pm")
                for key, K_s in (("sp", K_sp), ("bi", K_bi)):
                    for jb in range(NCH):
                        for ic in range(GCH):
                            nc.tensor.matmul(
                                pm[key][:, jb, :],
                                lhsT=K_s[:, ic, jb * 128:(jb + 1) * 128],
                                rhs=cur_q[:, ic, :],
                                start=(ic == 0), stop=(ic == GCH - 1))

                ocat_sp = work.tile([128, NCH, L], f16,
                                    name=f"ocat_sp_{it}", tag="ocat_sp")
                nc.scalar.copy(ocat_sp, pm["sp"][:])
                ocat_bi = work.tile([128, NCH, L], f16,
                                    name=f"ocat_bi_{it}", tag="ocat_bi")
                nc.vector.tensor_tensor(
                    ocat_bi, pm["bi"][:],
                    rn_bi.unsqueeze(-1).broadcast_to([128, NCH, L]),
                    op=ALU.mult)
                stp_sp = psum_mix.tile([L, SH], f16, name=f"stp_sp_{it}",
                                       tag="stp_sp")
                stp_bi = psum_mix.tile([L, SH], f16, name=f"stp_bi_{it}",
                                       tag="stp_bi")
                for c in range(NCH):
                    nc.tensor.transpose(
                        stp_sp[:, c * 128:(c + 1) * 128],
                        ocat_sp[:, c, :], ident_s[:, 0:128])
                    nc.tensor.transpose(
                        stp_bi[:, c * 128:(c + 1) * 128],
                        ocat_bi[:, c, :], ident_s[:, 0:128])
                nc.scalar.copy(sts[0:L, :], stp_sp[:])
                nc.vector.tensor_scalar_mul(sts[32:32 + L, :], stp_bi[:], 1.0)
                pmix = psum_mix.tile([L, SH], f32, name=f"pmix_{it}",
                                     tag="pmix")
                nc.tensor.matmul(pmix[:], lhsT=mixM_s[:], rhs=sts[:],
                                 start=True, stop=True)
                msgT = work.tile([L, SH], f16, name=f"msgT_{it}", tag="msgT")
                nc.scalar.copy(msgT[:, 0:SH // 2], pmix[:, 0:SH // 2])
                nc.vector.tensor_scalar_mul(
                    msgT[:, SH // 2:], pmix[:, SH // 2:], 1.0)
                pback = psum_mix.tile([128, NCH, 22], f16,
                                      name=f"pback_{it}", tag="pback")
                for c in range(NCH):
                    nc.tensor.transpose(
                        pback[:, c, 0:L], msgT[:, c * 128:(c + 1) * 128],
                        ident_s[0:L, 0:L])

                # cur = msg + unary
                newT = work.tile([128, NCH, L], f32, name=f"newT_{it}",
                                 tag="newT")
                nc.vector.tensor_tensor(
                    newT, pback[:, :, 0:L], unT_s, op=ALU.add)

                if it == NUM_ITERATIONS - 1:
                    nc.sync.dma_start(
                        out=outT_d.rearrange("(c p) l -> p c l", p=128),
                        in_=newT)
                    break

                # softmax over L (free axis) in (j x l) layout
                e = work.tile([128, NCH, L], f32, name=f"e_{it}", tag="e")
                nc.scalar.activation(e, newT, AF.Exp)
                ssum = work.tile([128, NCH], f32, name=f"ssum_{it}",
                                 tag="ssum")
                nc.vector.reduce_sum(ssum, e, axis=X_AXIS)
                rsum = work.tile([128, NCH], f32, name=f"rsum_{it}",
                                 tag="rsum")
                nc.vector.reciprocal(rsum, ssum)
                qTl = qpool.tile([128, NCH, L], f16, name=f"qTl_{it}",
                                 tag="qTl")
                nc.vector.tensor_tensor(
                    qTl, e, rsum.unsqueeze(-1).broadcast_to([128, NCH, L]),
                    op=ALU.mult)

                # all-gather qT through HBM
                qin = dram.tile([SH, L], f16, name=f"qin_{it}")
                nc.sync.dma_start(
                    out=qin.rearrange("(c p) l -> p c l", p=128), in_=qTl)
                qg = dram.tile([N, L], f16, name=f"qg_{it}",
                               addr_space="Shared")
                nc.gpsimd.collective_compute(
                    "AllGather", ALU.bypass, replica_groups=rg,
                    ins=[qin[:]], outs=[qg[:]])
                qg_v = qg.rearrange("(c p) l -> p c l", p=128)
                qTf = qpool.tile([128, GCH, L], f16, name=f"qTf_{it}",
                                 tag="qTf")
                for eng, lo, hi in ((nc.sync, 0, 11), (nc.scalar, 11, 22),
                                    (nc.gpsimd, 22, 32)):
                    eng.dma_start(out=qTf[:, lo:hi, :],
                                  in_=qg_v[:, lo:hi, :])
                cur_q = qTf

            it_ps.close()

    nc.compile()
    return nc


def _get_program():
    if "nc" not in _CACHE:
        _CACHE["nc"] = _build_program()
    return _CACHE["nc"]


def _host_prep(image, logits):
    img = np.asarray(image, np.float32)[0].reshape(C_IMG, N)
    unary = np.asarray(logits, np.float32)[0].reshape(L, N)

    zz, yy, xx = np.meshgrid(np.arange(D), np.arange(W), np.arange(H),
                             indexing="ij")
    pos = np.stack([zz, yy, xx]).reshape(3, N).astype(np.float32)

    feats_bi = np.concatenate([pos / ALPHA, img / BETA], axis=0)

    sq = np.sum(feats_bi.astype(np.float64) ** 2, axis=0)
    half = (-0.5 * sq[None, :]).astype(np.float32)
    one = np.ones((1, N), np.float32)
    a_bi = np.concatenate([feats_bi, one, half], 0).astype(np.float16)
    b_bi = np.concatenate([feats_bi, half, one], 0).astype(np.float16)
    return a_bi, b_bi, unary


def _sep_spatial():
    """Host tables for the separable spatial kernel (input-independent).

    The slice normalizer is folded in: gzt carries g1(zi-zj)/Sz(zj) and
    rnyx carries 1/(Sy(yj)*Sx(xj)) (applied to tyx after the exp).
    Returns (ayx, byx, gzt_per_core, rnyx)."""
    p = np.arange(128)
    ayx = np.zeros((4, 256), np.float32)
    for v in range(2):
        y_i = 8 * v + p // 16
        x_i = p % 16
        ayx[:, v * 128:(v + 1) * 128] = np.stack(
            [y_i, x_i, np.ones(128), -0.5 * (y_i ** 2 + x_i ** 2)])
    f = np.arange(256)
    y_j, x_j = f // 16, f % 16
    byx = np.stack([y_j, x_j, -0.5 * (y_j ** 2 + x_j ** 2),
                    np.ones(256)]).astype(np.float32)

    g1 = lambda d: np.exp(-0.5 * (d.astype(np.float64) / GAMMA) ** 2)
    axis = np.arange(16)
    S = np.array([g1(axis - t).sum() for t in range(16)])  # (16,)

    rnyx = np.broadcast_to(
        (1.0 / (S[y_j] * S[x_j]))[None, :], (128, 256)).astype(np.float16)

    gzt = []
    for c in range(NCORES):
        z_i = np.arange(GCH) // 2  # (GCH,)
        gz = np.empty((128, GCH, 2), np.float32)
        for h in range(2):
            z_j = 2 * c + h
            gz[:, :, h] = (g1(z_i - z_j) / S[z_j])[None, :]
        gzt.append(gz)
    return (ayx.astype(np.float16), byx.astype(np.float16), gzt, rnyx)


def _make_mixM(A_sp, A_bi):
    """[64, 21] stacked lhsT: rows 0:21 = A_sp^T, rows 32:53 = A_bi^T."""
    m = np.zeros((64, L), np.float32)
    m[0:L] = A_sp.T
    m[32:32 + L] = A_bi.T
    return m.astype(np.float16)


def _make_in_maps(image, logits, spatial_ker_weights, bilateral_ker_weights,
                  compatibility_matrix):
    a_bi, b_bi, unary = _host_prep(image, logits)

    A_sp = np.asarray(compatibility_matrix, np.float32) @ np.asarray(
        spatial_ker_weights, np.float32)
    A_bi = np.asarray(compatibility_matrix, np.float32) @ np.asarray(
        bilateral_ker_weights, np.float32)
    mixM = _make_mixM(A_sp, A_bi)
    ident = np.eye(128, dtype=np.float16)

    unaryT = np.ascontiguousarray(unary.T)  # (N, L)

    ayx, byx, gzt, rnyx = _sep_spatial()
    in_maps = []
    for c in range(NCORES):
        js = slice(c * SH, (c + 1) * SH)
        in_maps.append({
            "ayx": ayx,
            "byx": byx,
            "gzt": gzt[c],
            "rnyx": rnyx,
            "a_bi": a_bi,
            "b_bi": np.ascontiguousarray(b_bi[:, js]),
            "mixM": mixM,
            "ident": ident,
            "unT": np.ascontiguousarray(unaryT[js]),
            "unTf": unaryT.astype(np.float16),
        })
    return in_maps


def kernel(image, logits, spatial_ker_weights, bilateral_ker_weights,
           compatibility_matrix):
    in_maps = _make_in_maps(image, logits, spatial_ker_weights,
                            bilateral_ker_weights, compatibility_matrix)
    nc = _get_program()
    res = run_bass_kernel_spmd(nc, in_maps, core_ids=list(range(NCORES)))
    outT = np.concatenate([res.results[c]["outT"] for c in range(NCORES)],
                          axis=0)  # (N, L)
    return np.ascontiguousarray(outT.T).reshape(1, L, D, W, H).astype(
        np.float32)


if __name__ == "__main__":
    rng = np.random.default_rng(0)
    out = kernel(
        rng.random((1, C_IMG, D, W, H), np.float32),
        rng.standard_normal((1, L, D, W, H)).astype(np.float32),
        3.0 * np.eye(L, dtype=np.float32),
        5.0 * np.eye(L, dtype=np.float32),
        np.eye(L, dtype=np.float32),
    )
    print(out.shape, out.dtype, np.abs(out).max())


# revision 12
# speedup vs baseline: 1.0088x; 1.0088x over previous
"""CRF-RNN 3D dense-CRF mean-field kernel for Trainium2, sharded over 8 NeuronCores.

Strategy (column-sharded kernel matrices, K-stationary narrow GEMMs):
- Each core owns 512 columns (voxels j) of the two 4096x4096 Gaussian kernel
  matrices, stored fp16 in SBUF (8.4 MB total).
- Filtering GEMM is re-associated so the K chunk is the PE-stationary side and
  the narrow (21-wide) q tile is the moving side: out[j,l] += K[i,j]^T q[i,l]
  accumulated over 32 i-chunks into PSUM, directly in (voxel x label) layout.
  8 sequential accumulation chains (2 kernels x 4 j-blocks) per iteration.
- The bilateral slice-normalization 1/sum_i K[i,j] is computed by feeding a
  ones vector through the same K-stationary path (1-wide outputs), landing
  per-partition with no DRAM bounce; the spatial normalizer is folded into
  the separable build tables on the host (1/Sz into gzt, 1/(Sy*Sx) scaling
  tyx after the exp), so K_sp comes out pre-normalized.
- Label mixing (A = C@W per kernel) happens post-GEMM on own columns only:
  per-chunk PE transposes stack both kernels' (j x l) outputs as (l x j) rows
  0:21 / 32:53 of a zero-padded [64, 512] tile, one stacked matmul applies
  [A_sp^T; A_bi^T] and sums the two kernels, and 4 small transposes return
  msg to (j x l); the bilateral norm rides the psum->sbuf copy as a
  broadcast multiply.
- Iteration 1's softmax runs on the full replicated fp16 unary locally, and
  its GEMM interleaves with the kernel build (per-chunk dependencies), so no
  all-gather is needed before iteration 2.
- Iterations 2-5: softmax in (j x l) layout (ACT exp + DVE reduce/reciprocal/
  broadcast-scale), AllGather of qT (4096x21 fp16) through HBM, gather DMAs
  spread across DGE queues.
"""

import os
from contextlib import ExitStack
import sys

sys.path.insert(0, "/opt/trn_rl_repo")

import numpy as np

import concourse.bass as bass
from concourse.bass import _add_dep_helper
import concourse.tile as tile
from concourse import bacc, mybir
from concourse.bass_utils import run_bass_kernel_spmd

ALPHA, BETA, GAMMA = 67.0, 3.0, 1.0
NUM_ITERATIONS = 5
L = 21
C_IMG = 3
D = W = H = 16
N = D * W * H           # 4096
NCORES = 8
SH = N // NCORES        # 512 columns per core
NCH = SH // 128         # 4 local chunks (j-blocks)
GCH = N // 128          # 32 global chunks
FBI = 6 + 2             # bilateral features + [ones, -0.5|f|^2] augmentation

f32 = mybir.dt.float32
f16 = mybir.dt.float16
AF = mybir.ActivationFunctionType
ALU = mybir.AluOpType
X_AXIS = mybir.AxisListType.X

_CACHE = {}


def _build_program():
    """Emit the SPMD Bass program (identical for all 8 cores)."""
    nc = bacc.Bacc("TRN2", target_bir_lowering=False, debug=False,
                   num_devices=NCORES)

    ayx_d = nc.dram_tensor("ayx", [4, 256], f16, kind="ExternalInput").ap()
    byx_d = nc.dram_tensor("byx", [4, 256], f16, kind="ExternalInput").ap()
    gzt_d = nc.dram_tensor("gzt", [128, GCH, 2], f32, kind="ExternalInput").ap()
    rnyx_d = nc.dram_tensor("rnyx", [128, 256], f16, kind="ExternalInput").ap()
    a_bi_d = nc.dram_tensor("a_bi", [FBI, N], f16, kind="ExternalInput").ap()
    b_bi_d = nc.dram_tensor("b_bi", [FBI, SH], f16, kind="ExternalInput").ap()
    mixM_d = nc.dram_tensor("mixM", [64, L], f16, kind="ExternalInput").ap()
    ident_d = nc.dram_tensor("ident", [128, 128], f16,
                             kind="ExternalInput").ap()
    unT_d = nc.dram_tensor("unT", [SH, L], f32, kind="ExternalInput").ap()
    unTf_d = nc.dram_tensor("unTf", [N, L], f16, kind="ExternalInput").ap()
    outT_d = nc.dram_tensor("outT", [SH, L], f32, kind="ExternalOutput").ap()

    rg = [list(range(NCORES))]

    with tile.TileContext(nc) as tc:
        with (
            tc.tile_pool(name="const", bufs=1) as const,
            tc.tile_pool(name="kbig", bufs=1) as kbig,
            tc.tile_pool(name="work", bufs=3) as work,
            tc.tile_pool(name="qpool", bufs=2) as qpool,
            tc.tile_pool(name="dram", bufs=1, space="DRAM") as dram,
        ):
            # ---- load constants/input to SBUF ----
            # a_bi split so the first build matmuls start early
            a_bi_s = const.tile([FBI, N], f16)
            dmae = [nc.sync, nc.scalar, nc.gpsimd, nc.sync]
            for s in range(4):
                dmae[s].dma_start(out=a_bi_s[:, s * 1024:(s + 1) * 1024],
                                  in_=a_bi_d[:, s * 1024:(s + 1) * 1024])
            b_bi_s = const.tile([FBI, SH], f16)
            nc.scalar.dma_start(out=b_bi_s, in_=b_bi_d)
            ayx_s = const.tile([4, 256], f16)
            nc.sync.dma_start(out=ayx_s, in_=ayx_d)
            byx_s = const.tile([4, 256], f16)
            nc.sync.dma_start(out=byx_s, in_=byx_d)
            gzt_s = const.tile([128, GCH, 2], f32)
            nc.gpsimd.dma_start(out=gzt_s, in_=gzt_d)
            rnyx_s = const.tile([128, 256], f16)
            nc.sync.dma_start(out=rnyx_s, in_=rnyx_d)
            mixM_s = const.tile([64, L], f16)
            nc.scalar.dma_start(out=mixM_s, in_=mixM_d)
            ident_s = const.tile([128, 128], f16)
            nc.gpsimd.dma_start(out=ident_s, in_=ident_d)
            unT_s = const.tile([128, NCH, L], f32)
            nc.sync.dma_start(
                out=unT_s, in_=unT_d.rearrange("(c p) l -> p c l", p=128))
            # full transposed unary (all voxels, fp16): iteration 1's softmax
            # is computed locally on every core, skipping the first all-gather
            unTf_s = const.tile([128, GCH, L], f16)
            unTf_v = unTf_d.rearrange("(c p) l -> p c l", p=128)
            nc.sync.dma_start(out=unTf_s[:, 0:16, :], in_=unTf_v[:, 0:16, :])
            nc.scalar.dma_start(out=unTf_s[:, 16:32, :], in_=unTf_v[:, 16:32, :])
            ones_s = const.tile([128, 1], f16)
            nc.vector.memset(ones_s, 1.0)
            # persistent stacking tile for the mixing matmul; gap rows
            # (21:32, 53:64) are zeroed once and never written again
            sts = const.tile([64, SH], f16)
            nc.vector.memset(sts, 0.0)

            K_sp = kbig.tile([128, GCH, SH], f16)
            K_bi = kbig.tile([128, GCH, SH], f16)

            # ---- iteration 1 softmax on replicated unary (full voxel set) --
            ef = work.tile([128, GCH, L], f32, name="ef")
            nc.scalar.activation(ef, unTf_s, AF.Exp)
            sfull = work.tile([128, GCH], f32, name="sfull")
            nc.vector.reduce_sum(sfull, ef, axis=X_AXIS)
            rfull = work.tile([128, GCH], f32, name="rfull")
            nc.vector.reciprocal(rfull, sfull)
            qTf0 = qpool.tile([128, GCH, L], f16, name="qTf0", tag="qTf")
            qTf0_mult = nc.vector.tensor_tensor(
                qTf0, ef, rfull.unsqueeze(-1).broadcast_to([128, GCH, L]),
                op=ALU.mult)

            # ---- build kernel matrices ----
            build_ps = ExitStack()
            psum_build = build_ps.enter_context(
                tc.tile_pool(name="psum_build", bufs=2, space="PSUM"))

            # spatial: separable.  Tyx base tables via one matmul+exp, scaled
            # by the host-folded yx-normalizer, then K_sp = tyx * gz with the
            # z-normalizer folded into gzt on the host.
            ptyx = psum_build.tile([128, 2, 256], f32, tag="psb")
            for v in range(2):
                nc.tensor.matmul(
                    ptyx[:, v, :],
                    lhsT=ayx_s[:, v * 128:(v + 1) * 128],
                    rhs=byx_s[:],
                    start=True, stop=True)
            tyx = const.tile([128, 2, 256], f16)
            nc.scalar.activation(tyx, ptyx[:], AF.Exp)
            nc.vector.tensor_tensor(
                tyx, tyx, rnyx_s.unsqueeze(1).broadcast_to([128, 2, 256]),
                op=ALU.mult)
            # K_sp = tyx (x,y base) * gz (z factor): one broadcast
            # tensor_tensor per chunk parity, split across DVE and Pool
            for v, eng in ((0, nc.vector), (1, nc.gpsimd)):
                ksp_tt = eng.tensor_tensor(
                    K_sp[:, v::2, :].rearrange("p c (h f) -> p c h f", h=2),
                    tyx[:, v, :].unsqueeze(1).unsqueeze(1).broadcast_to(
                        [128, GCH // 2, 2, 256]),
                    gzt_s[:, v::2, :].unsqueeze(-1).broadcast_to(
                        [128, GCH // 2, 2, 256]),
                    op=ALU.mult)
                if eng is nc.vector:
                    # keep the iteration-1 softmax chain ahead of this big
                    # DVE op (head-of-line blocking otherwise)
                    _add_dep_helper(ksp_tt.ins, qTf0_mult.ins, sync=True,
                                    reason="ksp after iter1 softmax")

            # bilateral: dense augmented matmul + exp per 3-chunk group
            GRP = 3
            ic = 0
            while ic < GCH:
                g = min(GRP, GCH - ic)
                ps = psum_build.tile([128, GRP, SH], f32, tag="psb")
                for u in range(g):
                    nc.tensor.matmul(
                        ps[:, u, :],
                        lhsT=a_bi_s[:, (ic + u) * 128:(ic + u + 1) * 128],
                        rhs=b_bi_s[:],
                        start=True, stop=True)
                nc.scalar.activation(
                    K_bi[:, ic:ic + g, :], ps[:, 0:g, :], AF.Exp)
                ic += g
            build_ps.close()

            # bilateral norm via ones through the K-stationary path:
            # pn[j, 0] += K_bi[i, j]^T ones -> per-partition, no bounce
            rn_bi = const.tile([128, NCH], f32)
            with tc.tile_pool(name="psum_pn", bufs=1, space="PSUM") as psum_pn:
                pn = psum_pn.tile([128, NCH], f32)
                for jb in range(NCH):
                    for ic in range(GCH):
                        nc.tensor.matmul(
                            pn[:, jb:jb + 1],
                            lhsT=K_bi[:, ic, jb * 128:(jb + 1) * 128],
                            rhs=ones_s[:],
                            start=(ic == 0), stop=(ic == GCH - 1))
                nc.vector.reciprocal(rn_bi, pn[:])

            # ---- mean-field iterations ----
            it_ps = ExitStack()
            psum_gemm = it_ps.enter_context(
                tc.tile_pool(name="psum_gemm", bufs=1, space="PSUM"))
            psum_mix = it_ps.enter_context(
                tc.tile_pool(name="psum_mix", bufs=1, space="PSUM"))

            cur_q = qTf0
            for it in range(NUM_ITERATIONS):
                # filtering GEMMs: 8 sequential accumulation chains of
                # 32 chunk matmuls each, 21-wide moving side.  The mixing
                # copies/transposes for chain k are interleaved between
                # later chains so only the last chain's tail is exposed.
                pm = {}
                for key, K_s in (("sp", K_sp), ("bi", K_bi)):
                    pm[key] = psum_gemm.tile([128, NCH, L], f32,
                                             name=f"pm_{key}_{it}", tag="pm")
                for key, K_s in (("sp", K_sp), ("bi", K_bi)):
                    for jb in range(NCH):
                        for ic in range(GCH):
                            nc.tensor.matmul(
                                pm[key][:, jb, :],
                                lhsT=K_s[:, ic, jb * 128:(jb + 1) * 128],
                                rhs=cur_q[:, ic, :],
                                start=(ic == 0), stop=(ic == GCH - 1))

                ocat_sp = work.tile([128, NCH, L], f16,
                                    name=f"ocat_sp_{it}", tag="ocat_sp")
                nc.scalar.copy(ocat_sp, pm["sp"][:])
                ocat_bi = work.tile([128, NCH, L], f16,
                                    name=f"ocat_bi_{it}", tag="ocat_bi")
                nc.vector.tensor_tensor(
                    ocat_bi, pm["bi"][:],
                    rn_bi.unsqueeze(-1).broadcast_to([128, NCH, L]),
                    op=ALU.mult)
                stp_sp = psum_mix.tile([L, SH], f16, name=f"stp_sp_{it}",
                                       tag="stp_sp")
                stp_bi = psum_mix.tile([L, SH], f16, name=f"stp_bi_{it}",
                                       tag="stp_bi")
                for c in range(NCH):
                    nc.tensor.transpose(
                        stp_sp[:, c * 128:(c + 1) * 128],
                        ocat_sp[:, c, :], ident_s[:, 0:128])
                    nc.tensor.transpose(
                        stp_bi[:, c * 128:(c + 1) * 128],
                        ocat_bi[:, c, :], ident_s[:, 0:128])
                nc.scalar.copy(sts[0:L, :], stp_sp[:])
                nc.vector.tensor_scalar_mul(sts[32:32 + L, :], stp_bi[:], 1.0)
                pmix = psum_mix.tile([L, SH], f32, name=f"pmix_{it}",
                                     tag="pmix")
                nc.tensor.matmul(pmix[:], lhsT=mixM_s[:], rhs=sts[:],
                                 start=True, stop=True)
                msgT = work.tile([L, SH], f16, name=f"msgT_{it}", tag="msgT")
                nc.scalar.copy(msgT, pmix[:])
                pback = psum_mix.tile([128, NCH, 22], f16,
                                      name=f"pback_{it}", tag="pback")
                for c in range(NCH):
                    nc.tensor.transpose(
                        pback[:, c, 0:L], msgT[:, c * 128:(c + 1) * 128],
                        ident_s[0:L, 0:L])

                # cur = msg + unary
                newT = work.tile([128, NCH, L], f32, name=f"newT_{it}",
                                 tag="newT")
                nc.vector.tensor_tensor(
                    newT, pback[:, :, 0:L], unT_s, op=ALU.add)

                if it == NUM_ITERATIONS - 1:
                    nc.sync.dma_start(
                        out=outT_d.rearrange("(c p) l -> p c l", p=128),
                        in_=newT)
                    break

                # softmax over L (free axis) in (j x l) layout
                e = work.tile([128, NCH, L], f32, name=f"e_{it}", tag="e")
                nc.scalar.activation(e, newT, AF.Exp)
                ssum = work.tile([128, NCH], f32, name=f"ssum_{it}",
                                 tag="ssum")
                nc.vector.reduce_sum(ssum, e, axis=X_AXIS)
                rsum = work.tile([128, NCH], f32, name=f"rsum_{it}",
                                 tag="rsum")
                nc.vector.reciprocal(rsum, ssum)
                qTl = qpool.tile([128, NCH, L], f16, name=f"qTl_{it}",
                                 tag="qTl")
                nc.vector.tensor_tensor(
                    qTl, e, rsum.unsqueeze(-1).broadcast_to([128, NCH, L]),
                    op=ALU.mult)

                # all-gather qT through HBM
                qin = dram.tile([SH, L], f16, name=f"qin_{it}")
                nc.sync.dma_start(
                    out=qin.rearrange("(c p) l -> p c l", p=128), in_=qTl)
                qg = dram.tile([N, L], f16, name=f"qg_{it}",
                               addr_space="Shared")
                nc.gpsimd.collective_compute(
                    "AllGather", ALU.bypass, replica_groups=rg,
                    ins=[qin[:]], outs=[qg[:]])
                qg_v = qg.rearrange("(c p) l -> p c l", p=128)
                qTf = qpool.tile([128, GCH, L], f16, name=f"qTf_{it}",
                                 tag="qTf")
                for eng, lo, hi in ((nc.sync, 0, 11), (nc.scalar, 11, 22),
                                    (nc.gpsimd, 22, 32)):
                    eng.dma_start(out=qTf[:, lo:hi, :],
                                  in_=qg_v[:, lo:hi, :])
                cur_q = qTf

            it_ps.close()

    nc.compile()
    return nc


def _get_program():
    if "nc" not in _CACHE:
        _CACHE["nc"] = _build_program()
    return _CACHE["nc"]


def _host_prep(image, logits):
    img = np.asarray(image, np.float32)[0].reshape(C_IMG, N)
    unary = np.asarray(logits, np.float32)[0].reshape(L, N)

    zz, yy, xx = np.meshgrid(np.arange(D), np.arange(W), np.arange(H),
                             indexing="ij")
    pos = np.stack([zz, yy, xx]).reshape(3, N).astype(np.float32)

    feats_bi = np.concatenate([pos / ALPHA, img / BETA], axis=0)

    sq = np.sum(feats_bi.astype(np.float64) ** 2, axis=0)
    half = (-0.5 * sq[None, :]).astype(np.float32)
    one = np.ones((1, N), np.float32)
    a_bi = np.concatenate([feats_bi, one, half], 0).astype(np.float16)
    b_bi = np.concatenate([feats_bi, half, one], 0).astype(np.float16)
    return a_bi, b_bi, unary


def _sep_spatial():
    """Host tables for the separable spatial kernel (input-independent).

    The slice normalizer is folded in: gzt carries g1(zi-zj)/Sz(zj) and
    rnyx carries 1/(Sy(yj)*Sx(xj)) (applied to tyx after the exp).
    Returns (ayx, byx, gzt_per_core, rnyx)."""
    p = np.arange(128)
    ayx = np.zeros((4, 256), np.float32)
    for v in range(2):
        y_i = 8 * v + p // 16
        x_i = p % 16
        ayx[:, v * 128:(v + 1) * 128] = np.stack(
            [y_i, x_i, np.ones(128), -0.5 * (y_i ** 2 + x_i ** 2)])
    f = np.arange(256)
    y_j, x_j = f // 16, f % 16
    byx = np.stack([y_j, x_j, -0.5 * (y_j ** 2 + x_j ** 2),
                    np.ones(256)]).astype(np.float32)

    g1 = lambda d: np.exp(-0.5 * (d.astype(np.float64) / GAMMA) ** 2)
    axis = np.arange(16)
    S = np.array([g1(axis - t).sum() for t in range(16)])  # (16,)

    rnyx = np.broadcast_to(
        (1.0 / (S[y_j] * S[x_j]))[None, :], (128, 256)).astype(np.float16)

    gzt = []
    for c in range(NCORES):
        z_i = np.arange(GCH) // 2  # (GCH,)
        gz = np.empty((128, GCH, 2), np.float32)
        for h in range(2):
            z_j = 2 * c + h
            gz[:, :, h] = (g1(z_i - z_j) / S[z_j])[None, :]
        gzt.append(gz)
    return (ayx.astype(np.float16), byx.astype(np.float16), gzt, rnyx)


def _make_mixM(A_sp, A_bi):
    """[64, 21] stacked lhsT: rows 0:21 = A_sp^T, rows 32:53 = A_bi^T."""
    m = np.zeros((64, L), np.float32)
    m[0:L] = A_sp.T
    m[32:32 + L] = A_bi.T
    return m.astype(np.float16)


def _make_in_maps(image, logits, spatial_ker_weights, bilateral_ker_weights,
                  compatibility_matrix):
    a_bi, b_bi, unary = _host_prep(image, logits)

    A_sp = np.asarray(compatibility_matrix, np.float32) @ np.asarray(
        spatial_ker_weights, np.float32)
    A_bi = np.asarray(compatibility_matrix, np.float32) @ np.asarray(
        bilateral_ker_weights, np.float32)
    mixM = _make_mixM(A_sp, A_bi)
    ident = np.eye(128, dtype=np.float16)

    unaryT = np.ascontiguousarray(unary.T)  # (N, L)

    ayx, byx, gzt, rnyx = _sep_spatial()
    in_maps = []
    for c in range(NCORES):
        js = slice(c * SH, (c + 1) * SH)
        in_maps.append({
            "ayx": ayx,
            "byx": byx,
            "gzt": gzt[c],
            "rnyx": rnyx,
            "a_bi": a_bi,
            "b_bi": np.ascontiguousarray(b_bi[:, js]),
            "mixM": mixM,
            "ident": ident,
            "unT": np.ascontiguousarray(unaryT[js]),
            "unTf": unaryT.astype(np.float16),
        })
    return in_maps


def kernel(image, logits, spatial_ker_weights, bilateral_ker_weights,
           compatibility_matrix):
    in_maps = _make_in_maps(image, logits, spatial_ker_weights,
                            bilateral_ker_weights, compatibility_matrix)
    nc = _get_program()
    res = run_bass_kernel_spmd(nc, in_maps, core_ids=list(range(NCORES)))
    outT = np.concatenate([res.results[c]["outT"] for c in range(NCORES)],
                          axis=0)  # (N, L)
    return np.ascontiguousarray(outT.T).reshape(1, L, D, W, H).astype(
        np.float32)


if __name__ == "__main__":
    rng = np.random.default_rng(0)
    out = kernel(
        rng.random((1, C_IMG, D, W, H), np.float32),
        rng.standard_normal((1, L, D, W, H)).astype(np.float32),
        3.0 * np.eye(L, dtype=np.float32),
        5.0 * np.eye(L, dtype=np.float32),
        np.eye(L, dtype=np.float32),
    )
    print(out.shape, out.dtype, np.abs(out).max())


# revision 13
# speedup vs baseline: 1.0432x; 1.0341x over previous
"""CRF-RNN 3D dense-CRF mean-field kernel for Trainium2, sharded over 8 NeuronCores.

Strategy (column-sharded kernel matrices, K-stationary narrow GEMMs):
- Each core owns 512 columns (voxels j) of the two 4096x4096 Gaussian kernel
  matrices, stored fp16 in SBUF (8.4 MB total).
- Filtering GEMM is re-associated so the K chunk is the PE-stationary side and
  the narrow (21-wide) q tile is the moving side: out[j,l] += K[i,j]^T q[i,l]
  accumulated over 32 i-chunks into PSUM, directly in (voxel x label) layout.
  8 sequential accumulation chains (2 kernels x 4 j-blocks) per iteration.
- The bilateral slice-normalization 1/sum_i K[i,j] is computed by feeding a
  ones vector through the same K-stationary path (1-wide outputs), landing
  per-partition with no DRAM bounce; the spatial normalizer is folded into
  the separable build tables on the host (1/Sz into gzt, 1/(Sy*Sx) scaling
  tyx after the exp), so K_sp comes out pre-normalized.
- Label mixing (A = C@W per kernel) happens post-GEMM on own columns only:
  per-chunk PE transposes stack both kernels' (j x l) outputs as (l x j) rows
  0:21 / 32:53 of a zero-padded [64, 512] tile, one stacked matmul applies
  [A_sp^T; A_bi^T] and sums the two kernels, and 4 small transposes return
  msg to (j x l); the bilateral norm rides the psum->sbuf copy as a
  broadcast multiply.
- Iteration 1's softmax runs on the full replicated fp16 unary locally, and
  its GEMM interleaves with the kernel build (per-chunk dependencies), so no
  all-gather is needed before iteration 2.
- Iterations 2-5: softmax in (j x l) layout (ACT exp + DVE reduce/reciprocal/
  broadcast-scale), AllGather of qT (4096x21 fp16) through HBM, gather DMAs
  spread across DGE queues.
"""

import os
from contextlib import ExitStack
import sys

sys.path.insert(0, "/opt/trn_rl_repo")

import numpy as np

import concourse.bass as bass
from concourse.bass import _add_dep_helper
import concourse.tile as tile
from concourse import bacc, mybir
from concourse.bass_utils import run_bass_kernel_spmd

ALPHA, BETA, GAMMA = 67.0, 3.0, 1.0
NUM_ITERATIONS = 5
L = 21
C_IMG = 3
D = W = H = 16
N = D * W * H           # 4096
NCORES = 8
SH = N // NCORES        # 512 columns per core
NCH = SH // 128         # 4 local chunks (j-blocks)
GCH = N // 128          # 32 global chunks
FBI = 6 + 2             # bilateral features + [ones, -0.5|f|^2] augmentation

f32 = mybir.dt.float32
f16 = mybir.dt.float16
f8 = mybir.dt.float8e4
AF = mybir.ActivationFunctionType
ALU = mybir.AluOpType
X_AXIS = mybir.AxisListType.X

_CACHE = {}


def _build_program():
    """Emit the SPMD Bass program (identical for all 8 cores)."""
    nc = bacc.Bacc("TRN2", target_bir_lowering=False, debug=False,
                   num_devices=NCORES)

    ayx_d = nc.dram_tensor("ayx", [4, 256], f16, kind="ExternalInput").ap()
    byx_d = nc.dram_tensor("byx", [4, 256], f16, kind="ExternalInput").ap()
    gzt_d = nc.dram_tensor("gzt", [128, GCH, 2], f32, kind="ExternalInput").ap()
    rnyx_d = nc.dram_tensor("rnyx", [128, 256], f16, kind="ExternalInput").ap()
    a_bi_d = nc.dram_tensor("a_bi", [FBI, N], f16, kind="ExternalInput").ap()
    b_bi_d = nc.dram_tensor("b_bi", [FBI, SH], f16, kind="ExternalInput").ap()
    mixM_d = nc.dram_tensor("mixM", [64, L], f16, kind="ExternalInput").ap()
    ident_d = nc.dram_tensor("ident", [128, 128], f16,
                             kind="ExternalInput").ap()
    unT_d = nc.dram_tensor("unT", [SH, L], f32, kind="ExternalInput").ap()
    unTf_d = nc.dram_tensor("unTf", [N, L], f16, kind="ExternalInput").ap()
    outT_d = nc.dram_tensor("outT", [SH, L], f32, kind="ExternalOutput").ap()

    rg = [list(range(NCORES))]

    with tile.TileContext(nc) as tc:
        with (
            tc.tile_pool(name="const", bufs=1) as const,
            tc.tile_pool(name="kbig", bufs=1) as kbig,
            tc.tile_pool(name="work", bufs=3) as work,
            tc.tile_pool(name="qpool", bufs=2) as qpool,
            tc.tile_pool(name="dram", bufs=1, space="DRAM") as dram,
        ):
            # ---- load constants/input to SBUF ----
            # a_bi split so the first build matmuls start early
            a_bi_s = const.tile([FBI, N], f16)
            dmae = [nc.sync, nc.scalar, nc.gpsimd, nc.sync]
            for s in range(4):
                dmae[s].dma_start(out=a_bi_s[:, s * 1024:(s + 1) * 1024],
                                  in_=a_bi_d[:, s * 1024:(s + 1) * 1024])
            b_bi_s = const.tile([FBI, SH], f16)
            nc.scalar.dma_start(out=b_bi_s, in_=b_bi_d)
            ayx_s = const.tile([4, 256], f16)
            nc.sync.dma_start(out=ayx_s, in_=ayx_d)
            byx_s = const.tile([4, 256], f16)
            nc.sync.dma_start(out=byx_s, in_=byx_d)
            gzt_s = const.tile([128, GCH, 2], f32)
            nc.gpsimd.dma_start(out=gzt_s, in_=gzt_d)
            rnyx_s = const.tile([128, 256], f16)
            nc.sync.dma_start(out=rnyx_s, in_=rnyx_d)
            mixM_s = const.tile([64, L], f16)
            nc.scalar.dma_start(out=mixM_s, in_=mixM_d)
            ident_s = const.tile([128, 128], f16)
            nc.gpsimd.dma_start(out=ident_s, in_=ident_d)
            unT_s = const.tile([128, NCH, L], f32)
            nc.sync.dma_start(
                out=unT_s, in_=unT_d.rearrange("(c p) l -> p c l", p=128))
            # full transposed unary (all voxels, fp16): iteration 1's softmax
            # is computed locally on every core, skipping the first all-gather
            unTf_s = const.tile([128, GCH, L], f16)
            unTf_v = unTf_d.rearrange("(c p) l -> p c l", p=128)
            nc.sync.dma_start(out=unTf_s[:, 0:16, :], in_=unTf_v[:, 0:16, :])
            nc.scalar.dma_start(out=unTf_s[:, 16:32, :], in_=unTf_v[:, 16:32, :])
            ones_s = const.tile([128, 1], f16)
            nc.vector.memset(ones_s, 1.0)
            # persistent stacking tile for the mixing matmul; gap rows
            # (21:32, 53:64) are zeroed once and never written again
            sts = const.tile([64, SH], f16)
            nc.vector.memset(sts, 0.0)

            K_sp = kbig.tile([128, GCH, SH], f16)
            K_bi = kbig.tile([128, GCH, SH], f16)

            # ---- iteration 1 softmax on replicated unary (full voxel set) --
            ef = work.tile([128, GCH, L], f32, name="ef")
            nc.scalar.activation(ef, unTf_s, AF.Exp)
            sfull = work.tile([128, GCH], f32, name="sfull")
            nc.vector.reduce_sum(sfull, ef, axis=X_AXIS)
            rfull = work.tile([128, GCH], f32, name="rfull")
            nc.vector.reciprocal(rfull, sfull)
            qTf0 = qpool.tile([128, GCH, L], f16, name="qTf0", tag="qTf")
            qTf0_mult = nc.vector.tensor_tensor(
                qTf0, ef, rfull.unsqueeze(-1).broadcast_to([128, GCH, L]),
                op=ALU.mult)

            # ---- build kernel matrices ----
            build_ps = ExitStack()
            psum_build = build_ps.enter_context(
                tc.tile_pool(name="psum_build", bufs=2, space="PSUM"))

            # spatial: separable.  Tyx base tables via one matmul+exp, scaled
            # by the host-folded yx-normalizer, then K_sp = tyx * gz with the
            # z-normalizer folded into gzt on the host.
            ptyx = psum_build.tile([128, 2, 256], f32, tag="psb")
            for v in range(2):
                nc.tensor.matmul(
                    ptyx[:, v, :],
                    lhsT=ayx_s[:, v * 128:(v + 1) * 128],
                    rhs=byx_s[:],
                    start=True, stop=True)
            tyx = const.tile([128, 2, 256], f16)
            nc.scalar.activation(tyx, ptyx[:], AF.Exp)
            nc.vector.tensor_tensor(
                tyx, tyx, rnyx_s.unsqueeze(1).broadcast_to([128, 2, 256]),
                op=ALU.mult)
            # K_sp = tyx (x,y base) * gz (z factor): one broadcast
            # tensor_tensor per chunk parity, split across DVE and Pool
            for v, eng in ((0, nc.vector), (1, nc.gpsimd)):
                ksp_tt = eng.tensor_tensor(
                    K_sp[:, v::2, :].rearrange("p c (h f) -> p c h f", h=2),
                    tyx[:, v, :].unsqueeze(1).unsqueeze(1).broadcast_to(
                        [128, GCH // 2, 2, 256]),
                    gzt_s[:, v::2, :].unsqueeze(-1).broadcast_to(
                        [128, GCH // 2, 2, 256]),
                    op=ALU.mult)
                if eng is nc.vector:
                    # keep the iteration-1 softmax chain ahead of this big
                    # DVE op (head-of-line blocking otherwise)
                    _add_dep_helper(ksp_tt.ins, qTf0_mult.ins, sync=True,
                                    reason="ksp after iter1 softmax")

            # bilateral: dense augmented matmul + exp per 3-chunk group
            GRP = 3
            ic = 0
            while ic < GCH:
                g = min(GRP, GCH - ic)
                ps = psum_build.tile([128, GRP, SH], f32, tag="psb")
                for u in range(g):
                    nc.tensor.matmul(
                        ps[:, u, :],
                        lhsT=a_bi_s[:, (ic + u) * 128:(ic + u + 1) * 128],
                        rhs=b_bi_s[:],
                        start=True, stop=True)
                nc.scalar.activation(
                    K_bi[:, ic:ic + g, :], ps[:, 0:g, :], AF.Exp)
                ic += g
            build_ps.close()

            # bilateral norm via ones through the K-stationary path:
            # pn[j, 0] += K_bi[i, j]^T ones -> per-partition, no bounce
            rn_bi = const.tile([128, NCH], f32)
            with tc.tile_pool(name="psum_pn", bufs=1, space="PSUM") as psum_pn:
                pn = psum_pn.tile([128, NCH], f32)
                for jb in range(NCH):
                    for ic in range(GCH):
                        nc.tensor.matmul(
                            pn[:, jb:jb + 1],
                            lhsT=K_bi[:, ic, jb * 128:(jb + 1) * 128],
                            rhs=ones_s[:],
                            start=(ic == 0), stop=(ic == GCH - 1))
                nc.vector.reciprocal(rn_bi, pn[:])

            # ---- mean-field iterations ----
            it_ps = ExitStack()
            psum_gemm = it_ps.enter_context(
                tc.tile_pool(name="psum_gemm", bufs=1, space="PSUM"))
            psum_mix = it_ps.enter_context(
                tc.tile_pool(name="psum_mix", bufs=1, space="PSUM"))

            cur_q = qTf0
            for it in range(NUM_ITERATIONS):
                # filtering GEMMs: 8 sequential accumulation chains of
                # 32 chunk matmuls each, 21-wide moving side.  The mixing
                # copies/transposes for chain k are interleaved between
                # later chains so only the last chain's tail is exposed.
                pm = {}
                for key, K_s in (("sp", K_sp), ("bi", K_bi)):
                    pm[key] = psum_gemm.tile([128, NCH, L], f32,
                                             name=f"pm_{key}_{it}", tag="pm")
                for key, K_s in (("sp", K_sp), ("bi", K_bi)):
                    for jb in range(NCH):
                        for ic in range(GCH):
                            nc.tensor.matmul(
                                pm[key][:, jb, :],
                                lhsT=K_s[:, ic, jb * 128:(jb + 1) * 128],
                                rhs=cur_q[:, ic, :],
                                start=(ic == 0), stop=(ic == GCH - 1))

                ocat_sp = work.tile([128, NCH, L], f16,
                                    name=f"ocat_sp_{it}", tag="ocat_sp")
                nc.scalar.copy(ocat_sp, pm["sp"][:])
                ocat_bi = work.tile([128, NCH, L], f16,
                                    name=f"ocat_bi_{it}", tag="ocat_bi")
                nc.vector.tensor_tensor(
                    ocat_bi, pm["bi"][:],
                    rn_bi.unsqueeze(-1).broadcast_to([128, NCH, L]),
                    op=ALU.mult)
                stp_sp = psum_mix.tile([L, SH], f16, name=f"stp_sp_{it}",
                                       tag="stp_sp")
                stp_bi = psum_mix.tile([L, SH], f16, name=f"stp_bi_{it}",
                                       tag="stp_bi")
                for c in range(NCH):
                    nc.tensor.transpose(
                        stp_sp[:, c * 128:(c + 1) * 128],
                        ocat_sp[:, c, :], ident_s[:, 0:128])
                    nc.tensor.transpose(
                        stp_bi[:, c * 128:(c + 1) * 128],
                        ocat_bi[:, c, :], ident_s[:, 0:128])
                nc.scalar.copy(sts[0:L, :], stp_sp[:])
                nc.vector.tensor_scalar_mul(sts[32:32 + L, :], stp_bi[:], 1.0)
                pmix = psum_mix.tile([L, SH], f32, name=f"pmix_{it}",
                                     tag="pmix")
                nc.tensor.matmul(pmix[:], lhsT=mixM_s[:], rhs=sts[:],
                                 start=True, stop=True)
                msgT = work.tile([L, SH], f16, name=f"msgT_{it}", tag="msgT")
                nc.scalar.copy(msgT, pmix[:])
                pback = psum_mix.tile([128, NCH, 22], f16,
                                      name=f"pback_{it}", tag="pback")
                for c in range(NCH):
                    nc.tensor.transpose(
                        pback[:, c, 0:L], msgT[:, c * 128:(c + 1) * 128],
                        ident_s[0:L, 0:L])

                # cur = msg + unary
                newT = work.tile([128, NCH, L], f32, name=f"newT_{it}",
                                 tag="newT")
                nc.vector.tensor_tensor(
                    newT, pback[:, :, 0:L], unT_s, op=ALU.add)

                if it == NUM_ITERATIONS - 1:
                    nc.sync.dma_start(
                        out=outT_d.rearrange("(c p) l -> p c l", p=128),
                        in_=newT)
                    break

                # softmax over L (free axis) in (j x l) layout
                e = work.tile([128, NCH, L], f32, name=f"e_{it}", tag="e")
                nc.scalar.activation(e, newT, AF.Exp)
                ssum = work.tile([128, NCH], f32, name=f"ssum_{it}",
                                 tag="ssum")
                nc.vector.reduce_sum(ssum, e, axis=X_AXIS)
                rsum = work.tile([128, NCH], f32, name=f"rsum_{it}",
                                 tag="rsum")
                nc.vector.reciprocal(rsum, ssum)
                qTl = qpool.tile([128, NCH, L], f8, name=f"qTl_{it}",
                                 tag="qTl")
                with nc.allow_low_precision("fp8 all-gather payload"):
                    nc.vector.tensor_tensor(
                        qTl, e,
                        rsum.unsqueeze(-1).broadcast_to([128, NCH, L]),
                        op=ALU.mult)

                # all-gather qT through HBM as fp8 (halves the payload)
                qin = dram.tile([SH, L], f8, name=f"qin_{it}")
                nc.sync.dma_start(
                    out=qin.rearrange("(c p) l -> p c l", p=128), in_=qTl)
                qg = dram.tile([N, L], f8, name=f"qg_{it}",
                               addr_space="Shared")
                nc.gpsimd.collective_compute(
                    "AllGather", ALU.bypass, replica_groups=rg,
                    ins=[qin[:]], outs=[qg[:]])
                qg_v = qg.rearrange("(c p) l -> p c l", p=128)
                qTf8 = qpool.tile([128, GCH, L], f8, name=f"qTf8_{it}",
                                  tag="qTf8")
                for eng, lo, hi in ((nc.sync, 0, 11), (nc.scalar, 11, 22),
                                    (nc.gpsimd, 22, 32)):
                    eng.dma_start(out=qTf8[:, lo:hi, :],
                                  in_=qg_v[:, lo:hi, :])
                qTf = qpool.tile([128, GCH, L], f16, name=f"qTf_{it}",
                                 tag="qTf")
                nc.scalar.copy(qTf, qTf8[:])
                cur_q = qTf

            it_ps.close()

    nc.compile()
    return nc


def _get_program():
    if "nc" not in _CACHE:
        _CACHE["nc"] = _build_program()
    return _CACHE["nc"]


def _host_prep(image, logits):
    img = np.asarray(image, np.float32)[0].reshape(C_IMG, N)
    unary = np.asarray(logits, np.float32)[0].reshape(L, N)

    zz, yy, xx = np.meshgrid(np.arange(D), np.arange(W), np.arange(H),
                             indexing="ij")
    pos = np.stack([zz, yy, xx]).reshape(3, N).astype(np.float32)

    feats_bi = np.concatenate([pos / ALPHA, img / BETA], axis=0)

    sq = np.sum(feats_bi.astype(np.float64) ** 2, axis=0)
    half = (-0.5 * sq[None, :]).astype(np.float32)
    one = np.ones((1, N), np.float32)
    a_bi = np.concatenate([feats_bi, one, half], 0).astype(np.float16)
    b_bi = np.concatenate([feats_bi, half, one], 0).astype(np.float16)
    return a_bi, b_bi, unary


def _sep_spatial():
    """Host tables for the separable spatial kernel (input-independent).

    The slice normalizer is folded in: gzt carries g1(zi-zj)/Sz(zj) and
    rnyx carries 1/(Sy(yj)*Sx(xj)) (applied to tyx after the exp).
    Returns (ayx, byx, gzt_per_core, rnyx)."""
    p = np.arange(128)
    ayx = np.zeros((4, 256), np.float32)
    for v in range(2):
        y_i = 8 * v + p // 16
        x_i = p % 16
        ayx[:, v * 128:(v + 1) * 128] = np.stack(
            [y_i, x_i, np.ones(128), -0.5 * (y_i ** 2 + x_i ** 2)])
    f = np.arange(256)
    y_j, x_j = f // 16, f % 16
    byx = np.stack([y_j, x_j, -0.5 * (y_j ** 2 + x_j ** 2),
                    np.ones(256)]).astype(np.float32)

    g1 = lambda d: np.exp(-0.5 * (d.astype(np.float64) / GAMMA) ** 2)
    axis = np.arange(16)
    S = np.array([g1(axis - t).sum() for t in range(16)])  # (16,)

    rnyx = np.broadcast_to(
        (1.0 / (S[y_j] * S[x_j]))[None, :], (128, 256)).astype(np.float16)

    gzt = []
    for c in range(NCORES):
        z_i = np.arange(GCH) // 2  # (GCH,)
        gz = np.empty((128, GCH, 2), np.float32)
        for h in range(2):
            z_j = 2 * c + h
            gz[:, :, h] = (g1(z_i - z_j) / S[z_j])[None, :]
        gzt.append(gz)
    return (ayx.astype(np.float16), byx.astype(np.float16), gzt, rnyx)


def _make_mixM(A_sp, A_bi):
    """[64, 21] stacked lhsT: rows 0:21 = A_sp^T, rows 32:53 = A_bi^T."""
    m = np.zeros((64, L), np.float32)
    m[0:L] = A_sp.T
    m[32:32 + L] = A_bi.T
    return m.astype(np.float16)


def _make_in_maps(image, logits, spatial_ker_weights, bilateral_ker_weights,
                  compatibility_matrix):
    a_bi, b_bi, unary = _host_prep(image, logits)

    A_sp = np.asarray(compatibility_matrix, np.float32) @ np.asarray(
        spatial_ker_weights, np.float32)
    A_bi = np.asarray(compatibility_matrix, np.float32) @ np.asarray(
        bilateral_ker_weights, np.float32)
    mixM = _make_mixM(A_sp, A_bi)
    ident = np.eye(128, dtype=np.float16)

    unaryT = np.ascontiguousarray(unary.T)  # (N, L)

    ayx, byx, gzt, rnyx = _sep_spatial()
    in_maps = []
    for c in range(NCORES):
        js = slice(c * SH, (c + 1) * SH)
        in_maps.append({
            "ayx": ayx,
            "byx": byx,
            "gzt": gzt[c],
            "rnyx": rnyx,
            "a_bi": a_bi,
            "b_bi": np.ascontiguousarray(b_bi[:, js]),
            "mixM": mixM,
            "ident": ident,
            "unT": np.ascontiguousarray(unaryT[js]),
            "unTf": unaryT.astype(np.float16),
        })
    return in_maps


def kernel(image, logits, spatial_ker_weights, bilateral_ker_weights,
           compatibility_matrix):
    in_maps = _make_in_maps(image, logits, spatial_ker_weights,
                            bilateral_ker_weights, compatibility_matrix)
    nc = _get_program()
    res = run_bass_kernel_spmd(nc, in_maps, core_ids=list(range(NCORES)))
    outT = np.concatenate([res.results[c]["outT"] for c in range(NCORES)],
                          axis=0)  # (N, L)
    return np.ascontiguousarray(outT.T).reshape(1, L, D, W, H).astype(
        np.float32)


if __name__ == "__main__":
    rng = np.random.default_rng(0)
    out = kernel(
        rng.random((1, C_IMG, D, W, H), np.float32),
        rng.standard_normal((1, L, D, W, H)).astype(np.float32),
        3.0 * np.eye(L, dtype=np.float32),
        5.0 * np.eye(L, dtype=np.float32),
        np.eye(L, dtype=np.float32),
    )
    print(out.shape, out.dtype, np.abs(out).max())


# revision 15
# speedup vs baseline: 1.0904x; 1.0453x over previous
"""CRF-RNN 3D dense-CRF mean-field kernel for Trainium2, sharded over 8 NeuronCores.

Strategy (column-sharded kernel matrices, K-stationary narrow GEMMs):
- Each core owns 512 columns (voxels j) of the two 4096x4096 Gaussian kernel
  matrices, stored fp16 in SBUF (8.4 MB total).
- Filtering GEMM is re-associated so the K chunk is the PE-stationary side and
  the narrow (21-wide) q tile is the moving side: out[j,l] += K[i,j]^T q[i,l]
  accumulated over 32 i-chunks into PSUM, directly in (voxel x label) layout.
  8 sequential accumulation chains (2 kernels x 4 j-blocks) per iteration.
- The bilateral slice-normalization 1/sum_i K[i,j] is computed by feeding a
  ones vector through the same K-stationary path (1-wide outputs), landing
  per-partition with no DRAM bounce; the spatial normalizer is folded into
  the separable build tables on the host (1/Sz into gzt, 1/(Sy*Sx) scaling
  tyx after the exp), so K_sp comes out pre-normalized.
- Label mixing (A = C@W per kernel) happens post-GEMM on own columns only:
  per-chunk PE transposes stack both kernels' (j x l) outputs as (l x j) rows
  0:21 / 32:53 of a zero-padded [64, 512] tile, one stacked matmul applies
  [A_sp^T; A_bi^T] and sums the two kernels, and 4 small transposes return
  msg to (j x l); the bilateral norm rides the psum->sbuf copy as a
  broadcast multiply.
- Iteration 1's softmax runs on the full replicated fp16 unary locally, and
  its GEMM interleaves with the kernel build (per-chunk dependencies), so no
  all-gather is needed before iteration 2.
- Iterations 2-5: softmax in (j x l) layout (ACT exp + DVE reduce/reciprocal/
  broadcast-scale), AllGather of qT (4096x21 fp16) through HBM, gather DMAs
  spread across DGE queues.
"""

import os
from contextlib import ExitStack
import sys

sys.path.insert(0, "/opt/trn_rl_repo")

import numpy as np

import concourse.bass as bass
from concourse.bass import _add_dep_helper
import concourse.tile as tile
from concourse import bacc, mybir
from concourse.bass_utils import run_bass_kernel_spmd

ALPHA, BETA, GAMMA = 67.0, 3.0, 1.0
NUM_ITERATIONS = 5
L = 21
C_IMG = 3
D = W = H = 16
N = D * W * H           # 4096
NCORES = 8
SH = N // NCORES        # 512 columns per core
NCH = SH // 128         # 4 local chunks (j-blocks)
GCH = N // 128          # 32 global chunks
FBI = 6 + 2             # bilateral features + [ones, -0.5|f|^2] augmentation

f32 = mybir.dt.float32
f16 = mybir.dt.float16
f8 = mybir.dt.float8e4
AF = mybir.ActivationFunctionType
ALU = mybir.AluOpType
X_AXIS = mybir.AxisListType.X

_CACHE = {}


def _build_program():
    """Emit the SPMD Bass program (identical for all 8 cores)."""
    nc = bacc.Bacc("TRN2", target_bir_lowering=False, debug=False,
                   num_devices=NCORES)

    ayx_d = nc.dram_tensor("ayx", [4, 256], f16, kind="ExternalInput").ap()
    byx_d = nc.dram_tensor("byx", [4, 256], f16, kind="ExternalInput").ap()
    gzt_d = nc.dram_tensor("gzt", [128, GCH, 2], f32, kind="ExternalInput").ap()
    rnyx_d = nc.dram_tensor("rnyx", [128, 256], f16, kind="ExternalInput").ap()
    a_bi_d = nc.dram_tensor("a_bi", [FBI, N], f16, kind="ExternalInput").ap()
    b_bi_d = nc.dram_tensor("b_bi", [FBI, SH], f16, kind="ExternalInput").ap()
    mixM_d = nc.dram_tensor("mixM", [64, L], f16, kind="ExternalInput").ap()
    ident_d = nc.dram_tensor("ident", [128, 128], f16,
                             kind="ExternalInput").ap()
    unT_d = nc.dram_tensor("unT", [SH, L], f32, kind="ExternalInput").ap()
    unTf_d = nc.dram_tensor("unTf", [N, L], f16, kind="ExternalInput").ap()
    outT_d = nc.dram_tensor("outT", [SH, L], f32, kind="ExternalOutput").ap()

    rg = [list(range(NCORES))]

    with tile.TileContext(nc) as tc:
        with (
            tc.tile_pool(name="const", bufs=1) as const,
            tc.tile_pool(name="kbig", bufs=1) as kbig,
            tc.tile_pool(name="work", bufs=3) as work,
            tc.tile_pool(name="qpool", bufs=2) as qpool,
            tc.tile_pool(name="dram", bufs=1, space="DRAM") as dram,
        ):
            # ---- load constants/input to SBUF ----
            # a_bi split so the first build matmuls start early
            a_bi_s = const.tile([FBI, N], f16)
            dmae = [nc.sync, nc.scalar, nc.gpsimd, nc.sync]
            for s in range(4):
                dmae[s].dma_start(out=a_bi_s[:, s * 1024:(s + 1) * 1024],
                                  in_=a_bi_d[:, s * 1024:(s + 1) * 1024])
            b_bi_s = const.tile([FBI, SH], f16)
            nc.scalar.dma_start(out=b_bi_s, in_=b_bi_d)
            ayx_s = const.tile([4, 256], f16)
            nc.sync.dma_start(out=ayx_s, in_=ayx_d)
            byx_s = const.tile([4, 256], f16)
            nc.sync.dma_start(out=byx_s, in_=byx_d)
            gzt_s = const.tile([128, GCH, 2], f32)
            nc.gpsimd.dma_start(out=gzt_s, in_=gzt_d)
            rnyx_s = const.tile([128, 256], f16)
            nc.sync.dma_start(out=rnyx_s, in_=rnyx_d)
            mixM_s = const.tile([64, L], f16)
            nc.scalar.dma_start(out=mixM_s, in_=mixM_d)
            ident_s = const.tile([128, 128], f16)
            nc.gpsimd.dma_start(out=ident_s, in_=ident_d)
            unT_s = const.tile([128, NCH, L], f32)
            nc.sync.dma_start(
                out=unT_s, in_=unT_d.rearrange("(c p) l -> p c l", p=128))
            # full transposed unary (all voxels, fp16): iteration 1's softmax
            # is computed locally on every core, skipping the first all-gather
            unTf_s = const.tile([128, GCH, L], f16)
            unTf_v = unTf_d.rearrange("(c p) l -> p c l", p=128)
            nc.sync.dma_start(out=unTf_s[:, 0:16, :], in_=unTf_v[:, 0:16, :])
            nc.scalar.dma_start(out=unTf_s[:, 16:32, :], in_=unTf_v[:, 16:32, :])
            ones_s = const.tile([128, 1], f16)
            nc.vector.memset(ones_s, 1.0)
            # persistent stacking tile for the mixing matmul; gap rows
            # (21:32, 53:64) are zeroed once and never written again
            sts = const.tile([64, SH], f16)
            nc.vector.memset(sts, 0.0)

            K_sp = kbig.tile([128, GCH, SH], f16)
            K_bi = kbig.tile([128, GCH, SH], f16)

            # ---- iteration 1 softmax on replicated unary (full voxel set) --
            ef = work.tile([128, GCH, L], f32, name="ef")
            nc.scalar.activation(ef, unTf_s, AF.Exp)
            sfull = work.tile([128, GCH], f32, name="sfull")
            nc.vector.reduce_sum(sfull, ef, axis=X_AXIS)
            rfull = work.tile([128, GCH], f32, name="rfull")
            nc.vector.reciprocal(rfull, sfull)
            qTf0 = qpool.tile([128, GCH, L], f16, name="qTf0", tag="qTf")
            qTf0_mult = nc.vector.tensor_tensor(
                qTf0, ef, rfull.unsqueeze(-1).broadcast_to([128, GCH, L]),
                op=ALU.mult)

            # ---- build kernel matrices ----
            build_ps = ExitStack()
            psum_build = build_ps.enter_context(
                tc.tile_pool(name="psum_build", bufs=2, space="PSUM"))

            # spatial: separable.  Tyx base tables via one matmul+exp, scaled
            # by the host-folded yx-normalizer, then K_sp = tyx * gz with the
            # z-normalizer folded into gzt on the host.
            ptyx = psum_build.tile([128, 2, 256], f32, tag="psb")
            for v in range(2):
                nc.tensor.matmul(
                    ptyx[:, v, :],
                    lhsT=ayx_s[:, v * 128:(v + 1) * 128],
                    rhs=byx_s[:],
                    start=True, stop=True)
            tyx = const.tile([128, 2, 256], f16)
            nc.scalar.activation(tyx, ptyx[:], AF.Exp)
            nc.vector.tensor_tensor(
                tyx, tyx, rnyx_s.unsqueeze(1).broadcast_to([128, 2, 256]),
                op=ALU.mult)
            # K_sp = tyx (x,y base) * gz (z factor): one broadcast
            # tensor_tensor per chunk parity, split across DVE and Pool
            for v, eng in ((0, nc.vector), (1, nc.gpsimd)):
                ksp_tt = eng.tensor_tensor(
                    K_sp[:, v::2, :].rearrange("p c (h f) -> p c h f", h=2),
                    tyx[:, v, :].unsqueeze(1).unsqueeze(1).broadcast_to(
                        [128, GCH // 2, 2, 256]),
                    gzt_s[:, v::2, :].unsqueeze(-1).broadcast_to(
                        [128, GCH // 2, 2, 256]),
                    op=ALU.mult)
                if eng is nc.vector:
                    # keep the iteration-1 softmax chain ahead of this big
                    # DVE op (head-of-line blocking otherwise)
                    _add_dep_helper(ksp_tt.ins, qTf0_mult.ins, sync=True,
                                    reason="ksp after iter1 softmax")

            # bilateral: dense augmented matmul + exp per 3-chunk group
            GRP = 3
            ic = 0
            while ic < GCH:
                g = min(GRP, GCH - ic)
                ps = psum_build.tile([128, GRP, SH], f32, tag="psb")
                for u in range(g):
                    nc.tensor.matmul(
                        ps[:, u, :],
                        lhsT=a_bi_s[:, (ic + u) * 128:(ic + u + 1) * 128],
                        rhs=b_bi_s[:],
                        start=True, stop=True)
                nc.scalar.activation(
                    K_bi[:, ic:ic + g, :], ps[:, 0:g, :], AF.Exp)
                ic += g
            build_ps.close()

            # bilateral norm via ones through the K-stationary path:
            # pn[j, 0] += K_bi[i, j]^T ones -> per-partition, no bounce
            rn_bi = const.tile([128, NCH], f32)
            with tc.tile_pool(name="psum_pn", bufs=1, space="PSUM") as psum_pn:
                pn = psum_pn.tile([128, NCH], f32)
                for jb in range(NCH):
                    for ic in range(GCH):
                        nc.tensor.matmul(
                            pn[:, jb:jb + 1],
                            lhsT=K_bi[:, ic, jb * 128:(jb + 1) * 128],
                            rhs=ones_s[:],
                            start=(ic == 0), stop=(ic == GCH - 1))
                nc.vector.reciprocal(rn_bi, pn[:])

            # ---- mean-field iterations ----
            it_ps = ExitStack()
            psum_gemm = it_ps.enter_context(
                tc.tile_pool(name="psum_gemm", bufs=1, space="PSUM"))
            psum_mix = it_ps.enter_context(
                tc.tile_pool(name="psum_mix", bufs=1, space="PSUM"))

            cur_q = qTf0
            for it in range(NUM_ITERATIONS):
                # filtering GEMMs: 8 sequential accumulation chains of
                # 32 chunk matmuls each, 21-wide moving side.  The mixing
                # copies/transposes for chain k are interleaved between
                # later chains so only the last chain's tail is exposed.
                pm = {}
                for key, K_s in (("sp", K_sp), ("bi", K_bi)):
                    pm[key] = psum_gemm.tile([128, NCH, L], f32,
                                             name=f"pm_{key}_{it}", tag="pm")
                for key, K_s in (("sp", K_sp), ("bi", K_bi)):
                    for jb in range(NCH):
                        for ic in range(GCH):
                            nc.tensor.matmul(
                                pm[key][:, jb, :],
                                lhsT=K_s[:, ic, jb * 128:(jb + 1) * 128],
                                rhs=cur_q[:, ic, :],
                                start=(ic == 0), stop=(ic == GCH - 1))

                ocat_sp = work.tile([128, NCH, L], f16,
                                    name=f"ocat_sp_{it}", tag="ocat_sp")
                nc.scalar.copy(ocat_sp, pm["sp"][:])
                ocat_bi = work.tile([128, NCH, L], f16,
                                    name=f"ocat_bi_{it}", tag="ocat_bi")
                nc.vector.tensor_tensor(
                    ocat_bi, pm["bi"][:],
                    rn_bi.unsqueeze(-1).broadcast_to([128, NCH, L]),
                    op=ALU.mult)
                stp_sp = psum_mix.tile([L, SH], f16, name=f"stp_sp_{it}",
                                       tag="stp_sp")
                stp_bi = psum_mix.tile([L, SH], f16, name=f"stp_bi_{it}",
                                       tag="stp_bi")
                for c in range(NCH):
                    nc.tensor.transpose(
                        stp_sp[:, c * 128:(c + 1) * 128],
                        ocat_sp[:, c, :], ident_s[:, 0:128])
                    nc.tensor.transpose(
                        stp_bi[:, c * 128:(c + 1) * 128],
                        ocat_bi[:, c, :], ident_s[:, 0:128])
                nc.scalar.copy(sts[0:L, :], stp_sp[:])
                nc.vector.tensor_scalar_mul(sts[32:32 + L, :], stp_bi[:], 1.0)
                pmix = psum_mix.tile([L, SH], f32, name=f"pmix_{it}",
                                     tag="pmix")
                nc.tensor.matmul(pmix[:], lhsT=mixM_s[:], rhs=sts[:],
                                 start=True, stop=True)
                msgT = work.tile([L, SH], f16, name=f"msgT_{it}", tag="msgT")
                nc.scalar.copy(msgT, pmix[:])
                pback = psum_mix.tile([128, NCH, 22], f16,
                                      name=f"pback_{it}", tag="pback")
                for c in range(NCH):
                    nc.tensor.transpose(
                        pback[:, c, 0:L], msgT[:, c * 128:(c + 1) * 128],
                        ident_s[0:L, 0:L])

                # cur = msg + unary
                newT = work.tile([128, NCH, L], f32, name=f"newT_{it}",
                                 tag="newT")
                nc.vector.tensor_tensor(
                    newT, pback[:, :, 0:L], unT_s, op=ALU.add)

                if it == NUM_ITERATIONS - 1:
                    nc.sync.dma_start(
                        out=outT_d.rearrange("(c p) l -> p c l", p=128),
                        in_=newT)
                    break

                # softmax over L (free axis) in (j x l) layout
                e = work.tile([128, NCH, L], f32, name=f"e_{it}", tag="e")
                nc.scalar.activation(e, newT, AF.Exp)
                ssum = work.tile([128, NCH], f32, name=f"ssum_{it}",
                                 tag="ssum")
                nc.vector.reduce_sum(ssum, e, axis=X_AXIS)
                rsum = work.tile([128, NCH], f32, name=f"rsum_{it}",
                                 tag="rsum")
                nc.vector.reciprocal(rsum, ssum)
                qTl = qpool.tile([128, NCH, L], f8, name=f"qTl_{it}",
                                 tag="qTl")
                with nc.allow_low_precision("fp8 all-gather payload"):
                    nc.vector.tensor_tensor(
                        qTl, e,
                        rsum.unsqueeze(-1).broadcast_to([128, NCH, L]),
                        op=ALU.mult)

                # all-gather qT through HBM as fp8 (halves the payload)
                qin = dram.tile([SH, L], f8, name=f"qin_{it}")
                nc.sync.dma_start(
                    out=qin.rearrange("(c p) l -> p c l", p=128), in_=qTl)
                qg = dram.tile([N, L], f8, name=f"qg_{it}",
                               addr_space="Shared")
                nc.gpsimd.collective_compute(
                    "AllGather", ALU.bypass, replica_groups=rg,
                    ins=[qin[:]], outs=[qg[:]])
                qg_v = qg.rearrange("(c p) l -> p c l", p=128)
                qTf = qpool.tile([128, GCH, L], f16, name=f"qTf_{it}",
                                 tag="qTf")
                # dma-cast f8 -> f16 during the gather-in (gpsimd-only
                # capability; it also sees the collective completion a
                # sem-propagation earlier than the other engines)
                for lo, hi in ((0, 16), (16, 32)):
                    nc.gpsimd.dma_start(out=qTf[:, lo:hi, :],
                                        in_=qg_v[:, lo:hi, :])
                cur_q = qTf

            it_ps.close()

    nc.compile()
    return nc


def _get_program():
    if "nc" not in _CACHE:
        _CACHE["nc"] = _build_program()
    return _CACHE["nc"]


def _host_prep(image, logits):
    img = np.asarray(image, np.float32)[0].reshape(C_IMG, N)
    unary = np.asarray(logits, np.float32)[0].reshape(L, N)

    zz, yy, xx = np.meshgrid(np.arange(D), np.arange(W), np.arange(H),
                             indexing="ij")
    pos = np.stack([zz, yy, xx]).reshape(3, N).astype(np.float32)

    feats_bi = np.concatenate([pos / ALPHA, img / BETA], axis=0)

    sq = np.sum(feats_bi.astype(np.float64) ** 2, axis=0)
    half = (-0.5 * sq[None, :]).astype(np.float32)
    one = np.ones((1, N), np.float32)
    a_bi = np.concatenate([feats_bi, one, half], 0).astype(np.float16)
    b_bi = np.concatenate([feats_bi, half, one], 0).astype(np.float16)
    return a_bi, b_bi, unary


def _sep_spatial():
    """Host tables for the separable spatial kernel (input-independent).

    The slice normalizer is folded in: gzt carries g1(zi-zj)/Sz(zj) and
    rnyx carries 1/(Sy(yj)*Sx(xj)) (applied to tyx after the exp).
    Returns (ayx, byx, gzt_per_core, rnyx)."""
    p = np.arange(128)
    ayx = np.zeros((4, 256), np.float32)
    for v in range(2):
        y_i = 8 * v + p // 16
        x_i = p % 16
        ayx[:, v * 128:(v + 1) * 128] = np.stack(
            [y_i, x_i, np.ones(128), -0.5 * (y_i ** 2 + x_i ** 2)])
    f = np.arange(256)
    y_j, x_j = f // 16, f % 16
    byx = np.stack([y_j, x_j, -0.5 * (y_j ** 2 + x_j ** 2),
                    np.ones(256)]).astype(np.float32)

    g1 = lambda d: np.exp(-0.5 * (d.astype(np.float64) / GAMMA) ** 2)
    axis = np.arange(16)
    S = np.array([g1(axis - t).sum() for t in range(16)])  # (16,)

    rnyx = np.broadcast_to(
        (1.0 / (S[y_j] * S[x_j]))[None, :], (128, 256)).astype(np.float16)

    gzt = []
    for c in range(NCORES):
        z_i = np.arange(GCH) // 2  # (GCH,)
        gz = np.empty((128, GCH, 2), np.float32)
        for h in range(2):
            z_j = 2 * c + h
            gz[:, :, h] = (g1(z_i - z_j) / S[z_j])[None, :]
        gzt.append(gz)
    return (ayx.astype(np.float16), byx.astype(np.float16), gzt, rnyx)


def _make_mixM(A_sp, A_bi):
    """[64, 21] stacked lhsT: rows 0:21 = A_sp^T, rows 32:53 = A_bi^T."""
    m = np.zeros((64, L), np.float32)
    m[0:L] = A_sp.T
    m[32:32 + L] = A_bi.T
    return m.astype(np.float16)


def _make_in_maps(image, logits, spatial_ker_weights, bilateral_ker_weights,
                  compatibility_matrix):
    a_bi, b_bi, unary = _host_prep(image, logits)

    A_sp = np.asarray(compatibility_matrix, np.float32) @ np.asarray(
        spatial_ker_weights, np.float32)
    A_bi = np.asarray(compatibility_matrix, np.float32) @ np.asarray(
        bilateral_ker_weights, np.float32)
    mixM = _make_mixM(A_sp, A_bi)
    ident = np.eye(128, dtype=np.float16)

    unaryT = np.ascontiguousarray(unary.T)  # (N, L)

    ayx, byx, gzt, rnyx = _sep_spatial()
    in_maps = []
    for c in range(NCORES):
        js = slice(c * SH, (c + 1) * SH)
        in_maps.append({
            "ayx": ayx,
            "byx": byx,
            "gzt": gzt[c],
            "rnyx": rnyx,
            "a_bi": a_bi,
            "b_bi": np.ascontiguousarray(b_bi[:, js]),
            "mixM": mixM,
            "ident": ident,
            "unT": np.ascontiguousarray(unaryT[js]),
            "unTf": unaryT.astype(np.float16),
        })
    return in_maps


def kernel(image, logits, spatial_ker_weights, bilateral_ker_weights,
           compatibility_matrix):
    in_maps = _make_in_maps(image, logits, spatial_ker_weights,
                            bilateral_ker_weights, compatibility_matrix)
    nc = _get_program()
    res = run_bass_kernel_spmd(nc, in_maps, core_ids=list(range(NCORES)))
    outT = np.concatenate([res.results[c]["outT"] for c in range(NCORES)],
                          axis=0)  # (N, L)
    return np.ascontiguousarray(outT.T).reshape(1, L, D, W, H).astype(
        np.float32)


if __name__ == "__main__":
    rng = np.random.default_rng(0)
    out = kernel(
        rng.random((1, C_IMG, D, W, H), np.float32),
        rng.standard_normal((1, L, D, W, H)).astype(np.float32),
        3.0 * np.eye(L, dtype=np.float32),
        5.0 * np.eye(L, dtype=np.float32),
        np.eye(L, dtype=np.float32),
    )
    print(out.shape, out.dtype, np.abs(out).max())
